# revision 43
# baseline (speedup 1.0000x reference)
"""CRF decode backward RNN cell (Viterbi backtrace) Trainium2 kernel.

Problem: T=256, B=4096, NUM_TAGS=128.
  state_{t+1}[b] = backpointers[t, b, state_t[b]]
  out[t, b]      = tags_float[t, b, state_t[b]]

Sharding: batch-parallel across 8 NeuronCores (512 batch rows each).
Per core layout: batch -> 4 groups of 128 partitions; tags (128) on the
free axis.  The per-step gather is one fused DVE op per group:
  scalar_tensor_tensor(out=scratch, in0=iota, scalar=state, in1=row,
                       op0=is_equal, op1=mult, accum_out=result)
i.e. sum_k (k == state) * row[k]  ==  row[state], exact in fp32.
"""

import os
import sys

import numpy as np

for _p in ("/opt/trn_rl_repo",):
    if os.path.isdir(_p) and _p not in sys.path:
        sys.path.insert(0, _p)

import concourse.bass as bass
import concourse.mybir as mybir
from concourse import bacc
from concourse.bass_utils import run_bass_kernel_spmd
from concourse.tile import TileContext

T, B, K = 256, 4096, 128
NCORES = 8
BC = B // NCORES  # 512 batch rows per core
G = BC // 128  # 4 partition groups per core
SPD = 4  # timesteps fetched per DMA
NCHUNK = T // SPD

_CACHE: dict = {}


GCH = 32  # timesteps per indirect-gather chunk
NGCH = T // GCH


def build_program() -> bass.Bass:
    nc = bacc.Bacc("TRN2", debug=False, enable_asserts=False)
    # Backpointer values are < 128, exact in bf16: halves DMA traffic and
    # enables the DVE 16-bit fast path for the chain ops.
    bp = nc.dram_tensor("bp", [T, BC, K], mybir.dt.bfloat16, kind="ExternalInput")
    tf = nc.dram_tensor("tf", [T, BC, K], mybir.dt.float32, kind="ExternalInput")
    init = nc.dram_tensor("init", [BC], mybir.dt.int32, kind="ExternalInput")
    # Output stays in SBUF-native layout [p, (t j)]; host un-permutes.
    out = nc.dram_tensor("out", [128, T * G], mybir.dt.float32, kind="ExternalOutput")

    # DRAM views: batch row b = j*128 + p  ->  partition p, group j.
    # (t j) merges because stride(t) = G * stride(j) in the flat tensor.
    bp_r = bp.ap().rearrange("t (j p) k -> p (t j) k", p=128)  # [128, T*G, K]
    tf_r = tf.ap().rearrange("t (j p) k -> p (t j) k", p=128)
    init_r = init.ap().rearrange("(j p) -> p j", p=128)  # [128, G]

    with TileContext(nc) as tc:
        with (
            tc.tile_pool(name="bp_pool", bufs=3) as bp_pool,
            tc.tile_pool(name="tf_pool", bufs=3) as tf_pool,
            tc.tile_pool(name="misc", bufs=1) as misc,
            tc.tile_pool(name="sink", bufs=4) as sink_pool,
        ):
            iota = misc.tile([128, K], mybir.dt.float32)
            nc.gpsimd.iota(
                iota[:],
                pattern=[[1, K]],
                base=0,
                channel_multiplier=0,
                allow_small_or_imprecise_dtypes=True,
            )
            iota_bf = misc.tile([128, K], mybir.dt.bfloat16)
            nc.gpsimd.iota(
                iota_bf[:],
                pattern=[[1, K]],
                base=0,
                channel_multiplier=0,
                allow_small_or_imprecise_dtypes=True,
            )
            init_i = misc.tile([128, G], mybir.dt.int32)
            nc.gpsimd.dma_start(init_i[:], init_r)

            # states[:, t, j] = state at step t (pre-gather); col 0 = init.
            states = misc.tile([128, T + 1, G], mybir.dt.float32)
            nc.vector.tensor_copy(out=states[:, 0, :], in_=init_i[:])

            vals = misc.tile([128, T, G], mybir.dt.float32)
            # Per-group scratch: keeps every WAW dep >= 8 ops old so the
            # high-water sync op leaves the gather ops wait-free.
            scratch = [
                misc.tile([128, K], mybir.dt.bfloat16, tag=f"scr{j}", name=f"scr{j}")
                for j in range(G)
            ]
            scratch2 = [
                misc.tile([128, K], mybir.dt.float32, tag=f"scr2{j}", name=f"scr2{j}")
                for j in range(G)
            ]
            # Sink copies absorb cross-engine semaphore waits cheaply.
            sink0 = sink_pool.tile([128, 1], mybir.dt.float32, tag="snk")
            nc.vector.tensor_copy(out=sink0[:], in_=iota[:, 0:1])

            for c in range(NCHUNK):
                rows = slice(c * SPD * G, (c + 1) * SPD * G)
                bp_t = bp_pool.tile([128, SPD * G, K], mybir.dt.bfloat16)
                nc.sync.dma_start(bp_t[:], bp_r[:, rows, :])
                tf_t = tf_pool.tile([128, SPD * G, K], mybir.dt.float32)
                nc.scalar.dma_start(tf_t[:], tf_r[:, rows, :])
                s_a = sink_pool.tile([128, 1], mybir.dt.float32, tag="snk")
                nc.vector.tensor_copy(out=s_a[:], in_=bp_t[:, 0, 0:1])

                for lt in range(SPD):
                    t = c * SPD + lt
                    # High-water sync: one tiny op that waits on the newest
                    # DVE tick, so the 8 gather ops below need no waits.
                    if t > 0:
                        s_h = sink_pool.tile([128, 1], mybir.dt.float32, tag="snk")
                        nc.vector.tensor_copy(out=s_h[:], in_=scratch2[G - 1][:, 0:1])
                    for j in range(G):
                        row = lt * G + j
                        # State chain on DVE (critical path).
                        nc.vector.scalar_tensor_tensor(
                            out=scratch[j][:],
                            in0=iota_bf[:],
                            scalar=states[:, t, j : j + 1],
                            in1=bp_t[:, row, :],
                            op0=mybir.AluOpType.is_equal,
                            op1=mybir.AluOpType.mult,
                            accum_out=states[:, t + 1, j : j + 1],
                        )
                        # Value gather (off the critical path).
                        nc.vector.scalar_tensor_tensor(
                            out=scratch2[j][:],
                            in0=iota[:],
                            scalar=states[:, t, j : j + 1],
                            in1=tf_t[:, row, :],
                            op0=mybir.AluOpType.is_equal,
                            op1=mybir.AluOpType.mult,
                            accum_out=vals[:, t, j : j + 1],
                        )

            nc.gpsimd.dma_start(out.ap(), vals[:].rearrange("p t j -> p (t j)"))
    nc.compile()
    return nc


def _get_program() -> bass.Bass:
    if "nc" not in _CACHE:
        _CACHE["nc"] = build_program()
    return _CACHE["nc"]


def run(tags_float, backpointers, init_state, trace=False):
    tags_float = np.ascontiguousarray(tags_float, dtype=np.float32)
    backpointers = np.ascontiguousarray(backpointers, dtype=np.int32)
    init_state = np.ascontiguousarray(init_state, dtype=np.int32)
    assert tags_float.shape == (T, B, K) and backpointers.shape == (T, B, K)
    assert init_state.shape == (B,)

    nc = _get_program()
    import ml_dtypes

    bp_bf = backpointers.astype(ml_dtypes.bfloat16)  # values < 128: lossless
    in_maps = []
    for ci in range(NCORES):
        sl = slice(ci * BC, (ci + 1) * BC)
        in_maps.append(
            {
                "bp": np.ascontiguousarray(bp_bf[:, sl, :]),
                "tf": np.ascontiguousarray(tags_float[:, sl, :]),
                "init": np.ascontiguousarray(init_state[sl]),
            }
        )
    res = run_bass_kernel_spmd(
        nc, in_maps, core_ids=list(range(NCORES)), trace=trace
    )
    outs = []
    for ci in range(NCORES):
        arr = res.results[ci]["out"]  # [128, T*G] in (p, (t j)) layout
        outs.append(
            np.transpose(arr.reshape(128, T, G), (1, 2, 0)).reshape(T, BC, 1)
        )
    full = np.concatenate(outs, axis=1)
    return full, res.exec_time_ns


def kernel(tags_float, backpointers, init_state):
    out, _ = run(tags_float, backpointers, init_state)
    return out


# revision 47
# speedup vs baseline: 1.0215x; 1.0215x over previous
"""CRF decode backward RNN cell (Viterbi backtrace) Trainium2 kernel.

Problem: T=256, B=4096, NUM_TAGS=128.
  state_{t+1}[b] = backpointers[t, b, state_t[b]]
  out[t, b]      = tags_float[t, b, state_t[b]]

Sharding: batch-parallel across 8 NeuronCores (512 batch rows each).
Per core layout: batch -> 4 groups of 128 partitions; tags (128) on the
free axis.  The per-step gather is one fused DVE op per group:
  scalar_tensor_tensor(out=scratch, in0=iota, scalar=state, in1=row,
                       op0=is_equal, op1=mult, accum_out=result)
i.e. sum_k (k == state) * row[k]  ==  row[state], exact in fp32.
"""

import os
import sys

import numpy as np

for _p in ("/opt/trn_rl_repo",):
    if os.path.isdir(_p) and _p not in sys.path:
        sys.path.insert(0, _p)

import concourse.bass as bass
import concourse.mybir as mybir
from concourse import bacc
from concourse.bass_utils import run_bass_kernel_spmd
from concourse.tile import TileContext

T, B, K = 256, 4096, 128
NCORES = 8
BC = B // NCORES  # 512 batch rows per core
G = BC // 128  # 4 partition groups per core
SPD = 4  # timesteps fetched per DMA
NCHUNK = T // SPD

_CACHE: dict = {}


GCH = 32  # timesteps per indirect-gather chunk
NGCH = T // GCH


def build_program() -> bass.Bass:
    nc = bacc.Bacc("TRN2", debug=False, enable_asserts=False)
    # Backpointer values are < 128, exact in bf16: halves DMA traffic and
    # enables the DVE 16-bit fast path for the chain ops.
    bp = nc.dram_tensor("bp", [T, BC, K], mybir.dt.bfloat16, kind="ExternalInput")
    tf = nc.dram_tensor("tf", [T, BC, K], mybir.dt.float32, kind="ExternalInput")
    init = nc.dram_tensor("init", [BC], mybir.dt.int32, kind="ExternalInput")
    # Output stays in SBUF-native layout [p, (t j)]; host un-permutes.
    out = nc.dram_tensor("out", [128, T * G], mybir.dt.float32, kind="ExternalOutput")

    # DRAM views: batch row b = j*128 + p  ->  partition p, group j.
    # (t j) merges because stride(t) = G * stride(j) in the flat tensor.
    bp_r = bp.ap().rearrange("t (j p) k -> p (t j) k", p=128)  # [128, T*G, K]
    tf_r = tf.ap().rearrange("t (j p) k -> p (t j) k", p=128)
    init_r = init.ap().rearrange("(j p) -> p j", p=128)  # [128, G]

    with TileContext(nc) as tc:
        with (
            tc.tile_pool(name="bp_pool", bufs=3) as bp_pool,
            tc.tile_pool(name="tf_pool", bufs=3) as tf_pool,
            tc.tile_pool(name="misc", bufs=1) as misc,
            tc.tile_pool(name="sink", bufs=4) as sink_pool,
        ):
            iota = misc.tile([128, K], mybir.dt.float32)
            nc.gpsimd.iota(
                iota[:],
                pattern=[[1, K]],
                base=0,
                channel_multiplier=0,
                allow_small_or_imprecise_dtypes=True,
            )
            iota_bf = misc.tile([128, K], mybir.dt.bfloat16)
            nc.gpsimd.iota(
                iota_bf[:],
                pattern=[[1, K]],
                base=0,
                channel_multiplier=0,
                allow_small_or_imprecise_dtypes=True,
            )
            init_i = misc.tile([128, G], mybir.dt.int32)
            nc.gpsimd.dma_start(init_i[:], init_r)

            # states[:, t, j] = state at step t (pre-gather); col 0 = init.
            states = misc.tile([128, T + 1, G], mybir.dt.float32)
            nc.vector.tensor_copy(out=states[:, 0, :], in_=init_i[:])

            vals = misc.tile([128, T, G], mybir.dt.float32)
            scratch = misc.tile([128, K], mybir.dt.bfloat16)
            scratch2 = misc.tile([128, K], mybir.dt.float32)
            # Sink copies absorb cross-engine semaphore waits cheaply.
            sink0 = sink_pool.tile([128, 1], mybir.dt.float32, tag="snk")
            nc.vector.tensor_copy(out=sink0[:], in_=iota[:, 0:1])

            for c in range(NCHUNK):
                rows = slice(c * SPD * G, (c + 1) * SPD * G)
                bp_t = bp_pool.tile([128, SPD * G, K], mybir.dt.bfloat16)
                nc.sync.dma_start(bp_t[:], bp_r[:, rows, :])
                tf_t = tf_pool.tile([128, SPD * G, K], mybir.dt.float32)
                nc.scalar.dma_start(tf_t[:], tf_r[:, rows, :])
                s_a = sink_pool.tile([128, 1], mybir.dt.float32, tag="snk")
                nc.vector.tensor_copy(out=s_a[:], in_=bp_t[:, 0, 0:1])

                for lt in range(SPD):
                    t = c * SPD + lt
                    for j in range(G):
                        row = lt * G + j
                        # State chain on DVE (critical path).
                        nc.vector.scalar_tensor_tensor(
                            out=scratch[:],
                            in0=iota_bf[:],
                            scalar=states[:, t, j : j + 1],
                            in1=bp_t[:, row, :],
                            op0=mybir.AluOpType.is_equal,
                            op1=mybir.AluOpType.mult,
                            accum_out=states[:, t + 1, j : j + 1],
                        )
                        # Value gather (off the critical path).
                        nc.vector.scalar_tensor_tensor(
                            out=scratch2[:],
                            in0=iota[:],
                            scalar=states[:, t, j : j + 1],
                            in1=tf_t[:, row, :],
                            op0=mybir.AluOpType.is_equal,
                            op1=mybir.AluOpType.mult,
                            accum_out=vals[:, t, j : j + 1],
                        )

            nc.gpsimd.dma_start(out.ap(), vals[:].rearrange("p t j -> p (t j)"))
    nc.compile()
    return nc


def _get_program() -> bass.Bass:
    if "nc" not in _CACHE:
        _CACHE["nc"] = build_program()
    return _CACHE["nc"]


def run(tags_float, backpointers, init_state, trace=False):
    tags_float = np.ascontiguousarray(tags_float, dtype=np.float32)
    backpointers = np.ascontiguousarray(backpointers, dtype=np.int32)
    init_state = np.ascontiguousarray(init_state, dtype=np.int32)
    assert tags_float.shape == (T, B, K) and backpointers.shape == (T, B, K)
    assert init_state.shape == (B,)

    nc = _get_program()
    import ml_dtypes

    bp_bf = backpointers.astype(ml_dtypes.bfloat16)  # values < 128: lossless
    in_maps = []
    for ci in range(NCORES):
        sl = slice(ci * BC, (ci + 1) * BC)
        in_maps.append(
            {
                "bp": np.ascontiguousarray(bp_bf[:, sl, :]),
                "tf": np.ascontiguousarray(tags_float[:, sl, :]),
                "init": np.ascontiguousarray(init_state[sl]),
            }
        )
    res = run_bass_kernel_spmd(
        nc, in_maps, core_ids=list(range(NCORES)), trace=trace
    )
    outs = []
    for ci in range(NCORES):
        arr = res.results[ci]["out"]  # [128, T*G] in (p, (t j)) layout
        outs.append(
            np.transpose(arr.reshape(128, T, G), (1, 2, 0)).reshape(T, BC, 1)
        )
    full = np.concatenate(outs, axis=1)
    return full, res.exec_time_ns


def kernel(tags_float, backpointers, init_state):
    out, _ = run(tags_float, backpointers, init_state)
    return out


# revision 48
# speedup vs baseline: 1.0220x; 1.0005x over previous
"""CRF decode backward RNN cell (Viterbi backtrace) Trainium2 kernel.

Problem: T=256, B=4096, NUM_TAGS=128.
  state_{t+1}[b] = backpointers[t, b, state_t[b]]
  out[t, b]      = tags_float[t, b, state_t[b]]

Sharding: batch-parallel across 8 NeuronCores (512 batch rows each).
Per core layout: batch -> 4 groups of 128 partitions; tags (128) on the
free axis.  The per-step gather is one fused DVE op per group:
  scalar_tensor_tensor(out=scratch, in0=iota, scalar=state, in1=row,
                       op0=is_equal, op1=mult, accum_out=result)
i.e. sum_k (k == state) * row[k]  ==  row[state], exact in fp32.
"""

import os
import sys

import numpy as np

for _p in ("/opt/trn_rl_repo",):
    if os.path.isdir(_p) and _p not in sys.path:
        sys.path.insert(0, _p)

import concourse.bass as bass
import concourse.mybir as mybir
from concourse import bacc
from concourse.bass_utils import run_bass_kernel_spmd
from concourse.tile import TileContext

T, B, K = 256, 4096, 128
NCORES = 8
BC = B // NCORES  # 512 batch rows per core
G = BC // 128  # 4 partition groups per core
SPD = 4  # timesteps fetched per DMA
NCHUNK = T // SPD

_CACHE: dict = {}


GCH = 32  # timesteps per indirect-gather chunk
NGCH = T // GCH


def build_program() -> bass.Bass:
    nc = bacc.Bacc("TRN2", debug=False, enable_asserts=False)
    # Backpointer values are < 128, exact in bf16: halves DMA traffic and
    # enables the DVE 16-bit fast path for the chain ops.
    bp = nc.dram_tensor("bp", [T, BC, K], mybir.dt.bfloat16, kind="ExternalInput")
    tf = nc.dram_tensor("tf", [T, BC, K], mybir.dt.float32, kind="ExternalInput")
    init = nc.dram_tensor("init", [BC], mybir.dt.int32, kind="ExternalInput")
    # Output stays in SBUF-native layout [p, (t j)]; host un-permutes.
    out = nc.dram_tensor("out", [128, T * G], mybir.dt.float32, kind="ExternalOutput")

    # DRAM views: batch row b = j*128 + p  ->  partition p, group j.
    # (t j) merges because stride(t) = G * stride(j) in the flat tensor.
    bp_r = bp.ap().rearrange("t (j p) k -> p (t j) k", p=128)  # [128, T*G, K]
    tf_r = tf.ap().rearrange("t (j p) k -> p (t j) k", p=128)
    init_r = init.ap().rearrange("(j p) -> p j", p=128)  # [128, G]

    with TileContext(nc) as tc:
        with (
            tc.tile_pool(name="bp_pool", bufs=3) as bp_pool,
            tc.tile_pool(name="tf_pool", bufs=3) as tf_pool,
            tc.tile_pool(name="misc", bufs=1) as misc,
            tc.tile_pool(name="sink", bufs=4) as sink_pool,
        ):
            iota = misc.tile([128, K], mybir.dt.float32)
            nc.gpsimd.iota(
                iota[:],
                pattern=[[1, K]],
                base=0,
                channel_multiplier=0,
                allow_small_or_imprecise_dtypes=True,
            )
            iota_bf = misc.tile([128, K], mybir.dt.bfloat16)
            nc.gpsimd.iota(
                iota_bf[:],
                pattern=[[1, K]],
                base=0,
                channel_multiplier=0,
                allow_small_or_imprecise_dtypes=True,
            )
            init_i = misc.tile([128, G], mybir.dt.int32)
            nc.gpsimd.dma_start(init_i[:], init_r)

            # states[:, t, j] = state at step t (pre-gather); col 0 = init.
            # bf16 is exact for tag values < 128 and keeps the chain op
            # fully 16-bit for the DVE fast path.
            states = misc.tile([128, T + 1, G], mybir.dt.bfloat16)
            nc.vector.tensor_copy(out=states[:, 0, :], in_=init_i[:])

            vals = misc.tile([128, T, G], mybir.dt.float32)
            scratch = misc.tile([128, K], mybir.dt.bfloat16)
            scratch2 = misc.tile([128, K], mybir.dt.float32)
            # Sink copies absorb cross-engine semaphore waits cheaply.
            sink0 = sink_pool.tile([128, 1], mybir.dt.float32, tag="snk")
            nc.vector.tensor_copy(out=sink0[:], in_=iota[:, 0:1])

            for c in range(NCHUNK):
                rows = slice(c * SPD * G, (c + 1) * SPD * G)
                bp_t = bp_pool.tile([128, SPD * G, K], mybir.dt.bfloat16)
                nc.sync.dma_start(bp_t[:], bp_r[:, rows, :])
                tf_t = tf_pool.tile([128, SPD * G, K], mybir.dt.float32)
                nc.scalar.dma_start(tf_t[:], tf_r[:, rows, :])
                s_a = sink_pool.tile([128, 1], mybir.dt.float32, tag="snk")
                nc.vector.tensor_copy(out=s_a[:], in_=bp_t[:, 0, 0:1])

                for lt in range(SPD):
                    t = c * SPD + lt
                    for j in range(G):
                        row = lt * G + j
                        # State chain on DVE (critical path).
                        nc.vector.scalar_tensor_tensor(
                            out=scratch[:],
                            in0=iota_bf[:],
                            scalar=states[:, t, j : j + 1],
                            in1=bp_t[:, row, :],
                            op0=mybir.AluOpType.is_equal,
                            op1=mybir.AluOpType.mult,
                            accum_out=states[:, t + 1, j : j + 1],
                        )
                        # Value gather (off the critical path).
                        nc.vector.scalar_tensor_tensor(
                            out=scratch2[:],
                            in0=iota[:],
                            scalar=states[:, t, j : j + 1],
                            in1=tf_t[:, row, :],
                            op0=mybir.AluOpType.is_equal,
                            op1=mybir.AluOpType.mult,
                            accum_out=vals[:, t, j : j + 1],
                        )

            nc.gpsimd.dma_start(out.ap(), vals[:].rearrange("p t j -> p (t j)"))
    nc.compile()
    return nc


def _get_program() -> bass.Bass:
    if "nc" not in _CACHE:
        _CACHE["nc"] = build_program()
    return _CACHE["nc"]


def run(tags_float, backpointers, init_state, trace=False):
    tags_float = np.ascontiguousarray(tags_float, dtype=np.float32)
    backpointers = np.ascontiguousarray(backpointers, dtype=np.int32)
    init_state = np.ascontiguousarray(init_state, dtype=np.int32)
    assert tags_float.shape == (T, B, K) and backpointers.shape == (T, B, K)
    assert init_state.shape == (B,)

    nc = _get_program()
    import ml_dtypes

    bp_bf = backpointers.astype(ml_dtypes.bfloat16)  # values < 128: lossless
    in_maps = []
    for ci in range(NCORES):
        sl = slice(ci * BC, (ci + 1) * BC)
        in_maps.append(
            {
                "bp": np.ascontiguousarray(bp_bf[:, sl, :]),
                "tf": np.ascontiguousarray(tags_float[:, sl, :]),
                "init": np.ascontiguousarray(init_state[sl]),
            }
        )
    res = run_bass_kernel_spmd(
        nc, in_maps, core_ids=list(range(NCORES)), trace=trace
    )
    outs = []
    for ci in range(NCORES):
        arr = res.results[ci]["out"]  # [128, T*G] in (p, (t j)) layout
        outs.append(
            np.transpose(arr.reshape(128, T, G), (1, 2, 0)).reshape(T, BC, 1)
        )
    full = np.concatenate(outs, axis=1)
    return full, res.exec_time_ns


def kernel(tags_float, backpointers, init_state):
    out, _ = run(tags_float, backpointers, init_state)
    return out
